# revision 1
# baseline (speedup 1.0000x reference)
"""Trainium2 Bass kernel for nn_CNNEncoder (hashed n-gram embedding + conv/GLU stack).

Strategy (8 NeuronCores, data-parallel over batch, 2 batches/core):
- Embedding gather: tokens of each batch are bucket-sorted by word length on
  the host, so each 128-token tile needs only max-count gather slots instead
  of 12.  Gathers are [128,1]-offset indirect DMAs from a stacked table
  [3*50005, 128] (3 n-gram tables + emb0 rows appended per order), with
  CCE-add accumulation directly into the destination tile, chains
  interleaved so the Q7 descriptor generator never stalls.
- Scale by 1/count, assemble [tok, 384], scatter rows (indirect DMA, dest
  offset = original token position) into HBM staging; xbar DMA-transpose
  (bf16) builds the [384, 2048] conv input stripe.
- Conv stack: 5 layers of K-shifted bf16 matmuls accumulating in PSUM;
  weight-norm scales computed on device; GLU via ACT sigmoid (bias fused) +
  DVE (a+bias)*sig; residual sqrt(0.5) folded into weight/bias scales so the
  residual is a pure bf16 add.  Final h = C^5 * h_tilde.
"""

import sys

sys.path.insert(0, "/opt/trn_rl_repo")

from contextlib import ExitStack

import numpy as np

import concourse.bass as bass
import concourse.tile as tile
from concourse import bacc, mybir
from concourse.bass_utils import run_bass_kernel_spmd

B, S, N, E, V, L, KC, LYR = 16, 2048, 3, 128, 50000, 12, 3, 5
W = E * N
C = 0.7071067811865476
NCORES = 8
BPC = B // NCORES           # batches per core
TILES = S // 128            # 16 token tiles per batch
VS = V + 1 + 4              # rows per order block in stacked table (incl emb0)
WAVE = 8                    # ranked tiles per gather wave (x3 orders = 12 chains)


def _host_prep(inputs):
    x = np.asarray(inputs["x"]).astype(np.int64)
    ids = np.asarray(inputs["ngram_ids"]).astype(np.int64)
    cnt = np.asarray(inputs["ngram_counts"]).astype(np.int64)
    emb0 = np.asarray(inputs["emb0"]).astype(np.float32)
    tables = np.asarray(inputs["tables"]).astype(np.float32)
    conv_v = np.asarray(inputs["conv_v"]).astype(np.float32)
    conv_g = np.asarray(inputs["conv_g"]).astype(np.float32)
    conv_b = np.asarray(inputs["conv_b"]).astype(np.float32)

    # stacked table [3*VS, 128]: order n rows at n*VS + id; emb0 rows at n*VS+V+1+x
    tab = np.zeros((3 * VS, E), dtype=np.float32)
    for n in range(N):
        tab[n * VS : n * VS + V + 1] = tables[n]
        tab[n * VS + V + 1 : (n + 1) * VS] = emb0[:, n * E : (n + 1) * E]

    # per (core,batch): sort tokens by total count (== wordlen surrogate)
    special = x < 4                                    # [B, S]
    cnt_eff = np.where(special[..., None], 1, cnt)     # [B, S, 3]
    totc = np.where(special, 1, cnt.sum(-1))           # sort key [B, S]
    perm = np.argsort(totc, axis=1, kind="stable")     # sorted order -> orig pos
    cnt_sorted = np.take_along_axis(cnt_eff, perm[..., None], axis=1)  # [B,S,3]

    # shared K structure: K[r][n] = max over all batches of count at last rank of tile r
    Ksh = np.zeros((TILES, N), dtype=np.int64)
    for r in range(TILES):
        Ksh[r] = cnt_sorted[:, (r + 1) * 128 - 1, :].max(axis=0)
    Ksh = np.clip(Ksh, 1, L)

    # flat gather row ids per (b, s_orig, n, l): pad -> n*VS (zero row);
    # special -> first slot n*VS+V+1+x, count 1
    rows = ids + (np.arange(N) * VS)[None, None, :, None]          # [B,S,3,12]
    mask = np.arange(L)[None, None, None, :] < cnt_eff[..., None]
    rows = np.where(mask, rows, (np.arange(N) * VS)[None, None, :, None])
    sp_rows = (np.arange(N) * VS)[None, :] + V + 1 + x[..., None]  # [B,S,3]
    rows[special] = (np.arange(N)[None, :, None] * VS)             # zero all slots
    rows[special, :, 0] = sp_rows[special]

    # device emission order must match kernel build: for bb, for wavebase,
    # for j, for (r in wave, n) with Ksh[r][n] > j -> one offset column
    per_core = []
    for c in range(NCORES):
        offcols, rcp, pos = [], [], []
        for bb in range(BPC):
            b = c * BPC + bb
            pm = perm[b]
            srows = rows[b][pm]            # [S, 3, 12] sorted token order
            for wb in range(0, TILES, WAVE):
                wtiles = range(wb, min(wb + WAVE, TILES))
                jmax = int(max(Ksh[r].max() for r in wtiles))
                for j in range(jmax):
                    for r in wtiles:
                        for n in range(N):
                            if Ksh[r][n] > j:
                                tokens = pm[r * 128 : (r + 1) * 128]
                                offcols.append(rows[b][tokens, n, j])
            for r in range(TILES):
                for n in range(N):
                    rcp.append(1.0 / cnt_sorted[b, r * 128 : (r + 1) * 128, n])
                pos.append(pm[r * 128 : (r + 1) * 128])
        per_core.append(
            dict(
                off=np.stack(offcols, axis=1).astype(np.int32),   # [128, ncols]
                rcp=np.stack(rcp, axis=1).astype(np.float32),     # [128, 2*16*3]
                pos=np.stack(pos, axis=1).astype(np.int32),       # [128, 2*16]
            )
        )

    # conv_v -> [LYR, 2, KC, 3, 128, 384]  (l, half, k, ci, i, o')
    wv = conv_v.reshape(LYR, 2, 384, 3, 128, KC)
    wv = np.ascontiguousarray(wv.transpose(0, 1, 5, 3, 4, 2))
    cb = np.ascontiguousarray(conv_b.reshape(LYR, 6, 128).transpose(2, 0, 1)).reshape(
        128, LYR * 6
    )
    cg = conv_g.reshape(1, LYR * KC)
    return tab, wv, cg, cb, per_core, Ksh, perm


def _build(Ksh, repeat=1):
    nc = bacc.Bacc("TRN2", target_bir_lowering=False, debug=False)
    ncols = 0
    for wb in range(0, TILES, WAVE):
        wtiles = range(wb, min(wb + WAVE, TILES))
        jmax = int(max(Ksh[r].max() for r in wtiles))
        for j in range(jmax):
            for r in wtiles:
                for n in range(N):
                    if Ksh[r][n] > j:
                        ncols += 1
    ncols *= BPC

    t_tab = nc.dram_tensor("tab", [3 * VS, E], mybir.dt.float32, kind="ExternalInput")
    t_off = nc.dram_tensor("off", [128, ncols], mybir.dt.int32, kind="ExternalInput")
    t_rcp = nc.dram_tensor("rcp", [128, BPC * TILES * N], mybir.dt.float32, kind="ExternalInput")
    t_pos = nc.dram_tensor("pos", [128, BPC * TILES], mybir.dt.int32, kind="ExternalInput")
    t_wv = nc.dram_tensor("wv", [LYR, 2, KC, 3, 128, 384], mybir.dt.float32, kind="ExternalInput")
    t_cg = nc.dram_tensor("cg", [1, LYR * KC], mybir.dt.float32, kind="ExternalInput")
    t_cb = nc.dram_tensor("cb", [128, LYR * 6], mybir.dt.float32, kind="ExternalInput")
    t_est = [
        nc.dram_tensor(f"e_st{i}", [S, W], mybir.dt.float32, kind="ExternalOutput")
        for i in range(BPC)
    ]
    t_ebst = [
        nc.dram_tensor(f"ebst{i}", [S, W], mybir.dt.bfloat16, kind="Internal")
        for i in range(BPC)
    ]
    t_h = nc.dram_tensor("h_out", [BPC, W, S], mybir.dt.float32, kind="ExternalOutput")

    HW_ = 2112  # stripe width: tokens at [32, 2080), halos at 31 / 2080

    from contextlib import nullcontext
    with tile.TileContext(nc) as tc, ExitStack() as ctx:
        consts = ctx.enter_context(tc.tile_pool(name="consts", bufs=1))
        gd = ctx.enter_context(tc.tile_pool(name="gd", bufs=3 * WAVE + 3))
        asmp = ctx.enter_context(tc.tile_pool(name="asmp", bufs=4))
        bfp = ctx.enter_context(tc.tile_pool(name="bfp", bufs=4))
        hstr = ctx.enter_context(tc.tile_pool(name="hstr", bufs=3))
        rawp = ctx.enter_context(tc.tile_pool(name="rawp", bufs=1))
        wtp = ctx.enter_context(tc.tile_pool(name="wtp", bufs=2))
        sqp = ctx.enter_context(tc.tile_pool(name="sqp", bufs=1))
        scp = ctx.enter_context(tc.tile_pool(name="scp", bufs=2))
        sgp = ctx.enter_context(tc.tile_pool(name="sgp", bufs=6))
        hop = ctx.enter_context(tc.tile_pool(name="hop", bufs=4))
        psc = ctx.enter_context(tc.tile_pool(name="psc", bufs=3, space="PSUM"))
        psm = ctx.enter_context(tc.tile_pool(name="psm", bufs=1, space="PSUM"))

        off_t = consts.tile([128, ncols], mybir.dt.int32)
        nc.sync.dma_start(off_t[:], t_off.ap())
        rcp_t = consts.tile([128, BPC * TILES * N], mybir.dt.float32)
        nc.sync.dma_start(rcp_t[:], t_rcp.ap())
        pos_t = consts.tile([128, BPC * TILES], mybir.dt.int32)
        nc.sync.dma_start(pos_t[:], t_pos.ap())
        cb_t = consts.tile([128, LYR * 6], mybir.dt.float32)
        nc.sync.dma_start(cb_t[:], t_cb.ap())
        cg_t = consts.tile([1, LYR * KC], mybir.dt.float32)
        nc.sync.dma_start(cg_t[:], t_cg.ap())
        ones = consts.tile([128, 128], mybir.dt.float32)
        nc.vector.memset(ones[:], 1.0)

        rep_ctx = tc.For_i(0, repeat, 1) if repeat > 1 else nullcontext()
        ctx.enter_context(rep_ctx)
        # ---------------- embedding phase (both batches) ----------------
        colidx = 0
        for bb in range(BPC):
            for wb in range(0, TILES, WAVE):
                wtiles = list(range(wb, min(wb + WAVE, TILES)))
                gts = {}
                for r in wtiles:
                    for n in range(N):
                        gts[(r, n)] = gd.tile(
                            [128, E], mybir.dt.float32, name=f"g_{bb}_{r}_{n}", tag="gd"
                        )
                jmax = int(max(Ksh[r].max() for r in wtiles))
                for j in range(jmax):
                    for r in wtiles:
                        for n in range(N):
                            if Ksh[r][n] > j:
                                nc.gpsimd.indirect_dma_start(
                                    out=gts[(r, n)][:],
                                    out_offset=None,
                                    in_=t_tab.ap(),
                                    in_offset=bass.IndirectOffsetOnAxis(
                                        ap=off_t[:, colidx : colidx + 1], axis=0
                                    ),
                                    compute_op=(
                                        mybir.AluOpType.bypass
                                        if j == 0
                                        else mybir.AluOpType.add
                                    ),
                                )
                                colidx += 1
                for r in wtiles:
                    asm = asmp.tile([128, W], mybir.dt.float32, name=f"a_{bb}_{r}", tag="asm")
                    for n in range(N):
                        col = (bb * TILES + r) * N + n
                        nc.vector.tensor_scalar_mul(
                            asm[:, n * E : (n + 1) * E],
                            gts[(r, n)][:],
                            rcp_t[:, col : col + 1],
                        )
                    bf = bfp.tile([128, W], mybir.dt.bfloat16, name=f"b_{bb}_{r}", tag="bf")
                    nc.vector.tensor_copy(bf[:], asm[:])
                    pcol = bb * TILES + r
                    nc.sync.dma_start(
                        t_est[bb].ap()[r * 128 : (r + 1) * 128, :], asm[:]
                    )
                    nc.gpsimd.indirect_dma_start(
                        out=t_ebst[bb].ap(),
                        out_offset=bass.IndirectOffsetOnAxis(
                            ap=pos_t[:, pcol : pcol + 1], axis=0
                        ),
                        in_=bf[:],
                        in_offset=None,
                    )
        assert colidx == ncols

        # ---------------- conv phase per batch ----------------
        for bb in range(BPC):
            h0 = hstr.tile([128, N, HW_], mybir.dt.bfloat16, name=f"h0_{bb}", tag="hs")
            nc.vector.memset(h0[:, :, 31:32], 0.0)
            nc.vector.memset(h0[:, :, 2080:2081], 0.0)
            for n in range(N):
                nc.sync.dma_start(
                    h0[:, n, 32:2080],
                    t_ebst[bb].ap()[:, n * E : (n + 1) * E],
                    transpose=True,
                )
            hcur = h0
            for l in range(LYR):
                # ---- weight prep ----
                raw = rawp.tile([128, 2, KC, 3, 384], mybir.dt.float32, name=f"rw{bb}{l}", tag="raw")
                nc.sync.dma_start(
                    raw[:],
                    t_wv.ap()[l].rearrange("h k c i o -> i h k c o"),
                )
                ssq = scp.tile([128, KC], mybir.dt.float32, name=f"sq{bb}{l}", tag="ssq")
                sq = sqp.tile([128, 2, 3, 384], mybir.dt.float32, name=f"s2{bb}{l}", tag="sq")
                for k in range(KC):
                    nc.scalar.square(sq[:], raw[:, :, k, :, :])
                    nc.vector.tensor_reduce(
                        ssq[:, k : k + 1], sq[:],
                        axis=mybir.AxisListType.XYZ, op=mybir.AluOpType.add,
                    )
                pnrm = psm.tile([1, KC], mybir.dt.float32, space="PSUM", name=f"pn{bb}{l}", tag="pn")
                nc.tensor.matmul(pnrm[:], ones[:, 0:1], ssq[:], start=True, stop=True)
                nrm = scp.tile([1, 8], mybir.dt.float32, name=f"nr{bb}{l}", tag="nrm")
                nc.scalar.sqrt(nrm[0:1, 0:KC], pnrm[:])
                nc.vector.reciprocal(nrm[0:1, 3:6], nrm[0:1, 0:KC])
                nc.vector.tensor_mul(
                    nrm[0:1, 0:KC], nrm[0:1, 3:6], cg_t[0:1, l * KC : (l + 1) * KC]
                )
                sab = scp.tile([1, 6], mybir.dt.float32, name=f"sb{bb}{l}", tag="sab")
                nc.vector.tensor_scalar_mul(sab[0:1, 0:3], nrm[0:1, 0:KC], 1.0)
                nc.vector.tensor_scalar_mul(sab[0:1, 3:6], nrm[0:1, 0:KC], C**l)
                psb = psm.tile([128, 6], mybir.dt.float32, space="PSUM", name=f"pb{bb}{l}", tag="pb")
                nc.tensor.matmul(psb[:], ones[0:1, 0:128], sab[0:1, :], start=True, stop=True)
                sbc = scp.tile([128, 6], mybir.dt.float32, name=f"sc{bb}{l}", tag="sbc")
                nc.vector.tensor_copy(sbc[:], psb[:])
                wT = wtp.tile([128, 2, KC, 3, 384], mybir.dt.bfloat16, name=f"wt{bb}{l}", tag="wt")
                for h in range(2):
                    for k in range(KC):
                        nc.vector.tensor_scalar_mul(
                            wT[:, h, k, :, :],
                            raw[:, h, k, :, :],
                            sbc[:, h * KC + k : h * KC + k + 1],
                        )
                bae = scp.tile([128, KC], mybir.dt.float32, name=f"ba{bb}{l}", tag="bae")
                nc.vector.tensor_scalar_mul(
                    bae[:], cb_t[:, l * 6 : l * 6 + 3], C ** (-l)
                )

                hnext = (
                    hstr.tile([128, N, HW_], mybir.dt.bfloat16, name=f"h{bb}_{l + 1}", tag="hs")
                    if l < LYR - 1
                    else None
                )
                if hnext is not None:
                    nc.vector.memset(hnext[:, :, 31:32], 0.0)
                    nc.vector.memset(hnext[:, :, 2080:2081], 0.0)
                for pj in range(3):
                    for nt in range(4):
                        ps_a = psc.tile([128, 512], mybir.dt.float32, space="PSUM",
                                        name=f"pa{bb}{l}{pj}{nt}", tag="psa")
                        ps_b = psc.tile([128, 512], mybir.dt.float32, space="PSUM",
                                        name=f"pq{bb}{l}{pj}{nt}", tag="psb")
                        for ci in range(3):
                            for k in range(KC):
                                rhs = hcur[:, ci, 32 + nt * 512 + k - 1 : 32 + nt * 512 + k + 511]
                                st = ci == 0 and k == 0
                                sp = ci == 2 and k == KC - 1
                                nc.tensor.matmul(
                                    ps_a[:], wT[:, 0, k, ci, pj * 128 : (pj + 1) * 128],
                                    rhs, start=st, stop=sp,
                                )
                                nc.tensor.matmul(
                                    ps_b[:], wT[:, 1, k, ci, pj * 128 : (pj + 1) * 128],
                                    rhs, start=st, stop=sp,
                                )
                        sig = sgp.tile([128, 512], mybir.dt.bfloat16,
                                       name=f"sg{bb}{l}{pj}{nt}", tag="sig")
                        nc.scalar.activation(
                            sig[:], ps_b[:], mybir.ActivationFunctionType.Sigmoid,
                            bias=cb_t[:, l * 6 + 3 + pj : l * 6 + 4 + pj], scale=1.0,
                        )
                        if hnext is not None:
                            glu = sgp.tile([128, 512], mybir.dt.bfloat16,
                                           name=f"gl{bb}{l}{pj}{nt}", tag="glu")
                            nc.vector.scalar_tensor_tensor(
                                glu[:], ps_a[:], bae[:, pj : pj + 1], sig[:],
                                op0=mybir.AluOpType.add, op1=mybir.AluOpType.mult,
                            )
                            nc.vector.tensor_add(
                                hnext[:, pj, 32 + nt * 512 : 32 + (nt + 1) * 512],
                                glu[:],
                                hcur[:, pj, 32 + nt * 512 : 32 + (nt + 1) * 512],
                            )
                        else:
                            # last layer: h_out = C^5*(glu + hcur) computed in fp32
                            glu = sgp.tile([128, 512], mybir.dt.float32,
                                           name=f"gl{bb}{l}{pj}{nt}", tag="gluf")
                            nc.vector.scalar_tensor_tensor(
                                glu[:], ps_a[:], bae[:, pj : pj + 1], sig[:],
                                op0=mybir.AluOpType.add, op1=mybir.AluOpType.mult,
                            )
                            ho = hop.tile([128, 512], mybir.dt.float32, name=f"ho{bb}{pj}{nt}", tag="ho")
                            nc.vector.scalar_tensor_tensor(
                                ho[:], hcur[:, pj, 32 + nt * 512 : 32 + (nt + 1) * 512],
                                1.0, glu[:],
                                op0=mybir.AluOpType.mult, op1=mybir.AluOpType.add,
                            )
                            hs = hop.tile([128, 512], mybir.dt.float32,
                                          name=f"hs{bb}{pj}{nt}", tag="hsc")
                            nc.vector.tensor_scalar_mul(hs[:], ho[:], C**LYR)
                            nc.sync.dma_start(
                                t_h.ap()[bb][pj * 128 : (pj + 1) * 128,
                                             nt * 512 : (nt + 1) * 512],
                                hs[:],
                            )
                hcur = hnext if hnext is not None else hcur
    nc.compile()
    return nc


_CACHE = {}


def _run(inputs, trace=False, repeat=1):
    tab, wv, cg, cb, per_core, Ksh, perm = _host_prep(inputs)
    key = (Ksh.tobytes(), repeat)
    if key not in _CACHE:
        _CACHE[key] = _build(Ksh, repeat=repeat)
    nc = _CACHE[key]
    in_maps = [
        dict(tab=tab, off=pc["off"], rcp=pc["rcp"], pos=pc["pos"], wv=wv, cg=cg, cb=cb)
        for pc in per_core
    ]
    res = run_bass_kernel_spmd(nc, in_maps, core_ids=list(range(NCORES)), trace=trace)
    h = np.concatenate([r["h_out"] for r in res.results], axis=0)        # [16, 384, 2048]
    e = np.empty((B, W, S), dtype=np.float32)
    for c in range(NCORES):
        for i in range(BPC):
            b = c * BPC + i
            es = np.empty((S, W), dtype=np.float32)
            es[perm[b]] = res.results[c][f"e_st{i}"]     # unsort rows
            e[b] = es.T
    return (h.astype(np.float32), np.ascontiguousarray(e).astype(np.float32)), res


def kernel(**inputs):
    out, _ = _run(inputs)
    return out



# revision 2
# speedup vs baseline: 10365.3911x; 10365.3911x over previous
"""Trainium2 Bass kernel for nn_CNNEncoder (hashed n-gram embedding + conv/GLU stack).

Strategy (8 NeuronCores, data-parallel over batch, 2 batches/core):
- Embedding gather: tokens of each batch are bucket-sorted by word length on
  the host.  All 48 (tile, order) pairs of a batch are sorted by max-count
  descending; gather step j issues ONE multi-column indirect DMA covering the
  prefix of pairs still active at slot j (CCE-add accumulation), alternating
  between two accumulator tiles A/B so descriptor generation overlaps the
  previous transfer.  This amortizes the ~1us SWDGE fixed cost over ~6000
  descriptors instead of 128.
- A+B combine, scale by 1/count, assemble [tok, 384] fp32 (e output) and bf16
  (conv input); ONE indirect scatter per batch places bf16 rows at original
  token positions in HBM staging; xbar DMA-transpose builds the [384, 2048]
  conv input stripe.
- Conv stack: weight-norm scales/biases folded on the HOST into bf16 weights
  loaded once and kept resident in SBUF.  5 layers of K-shifted bf16 matmuls
  accumulating in PSUM; GLU via ACT sigmoid (bias fused) + DVE (a+bias)*sig;
  residual sqrt(0.5) folded into weight/bias scales so the residual is a pure
  bf16 add.  Final h = C^5 * h_tilde.
"""

import sys

sys.path.insert(0, "/opt/trn_rl_repo")

from contextlib import ExitStack

import numpy as np

import concourse.bass as bass
import concourse.tile as tile
from concourse import bacc, mybir
from concourse.bass_utils import run_bass_kernel_spmd

B, S, N, E, V, L, KC, LYR = 16, 2048, 3, 128, 50000, 12, 3, 5
W = E * N
C = 0.7071067811865476
NCORES = 8
BPC = B // NCORES           # batches per core
TILES = S // 128            # 16 token tiles per batch
VS = V + 1 + 4              # rows per order block in stacked table (incl emb0)
NP = TILES * N              # 48 (tile, order) pairs per batch


def _pair_schedule(Ksh):
    """Sorted pair order + per-j gather prefix sizes (shared by host & build)."""
    pairs = sorted(
        [(r, n) for r in range(TILES) for n in range(N)],
        key=lambda rn: (-Ksh[rn[0]][rn[1]], rn[0], rn[1]),
    )
    jmax = int(max(Ksh[r][n] for r, n in pairs))
    prefix = []
    for j in range(jmax):
        P = sum(1 for r, n in pairs if Ksh[r][n] > j)
        if j <= 1:
            P = NP  # pad so both A (j=0) and B (j=1) get full-width bypass
        prefix.append(P)
    return pairs, jmax, prefix


def _host_prep(inputs):
    x = np.asarray(inputs["x"]).astype(np.int64)
    ids = np.asarray(inputs["ngram_ids"]).astype(np.int64)
    cnt = np.asarray(inputs["ngram_counts"]).astype(np.int64)
    emb0 = np.asarray(inputs["emb0"]).astype(np.float32)
    tables = np.asarray(inputs["tables"]).astype(np.float32)
    conv_v = np.asarray(inputs["conv_v"]).astype(np.float64)
    conv_g = np.asarray(inputs["conv_g"]).astype(np.float64)
    conv_b = np.asarray(inputs["conv_b"]).astype(np.float64)

    # stacked table [3*VS, 128]: order n rows at n*VS + id; emb0 rows at n*VS+V+1+x
    tab = np.zeros((3 * VS, E), dtype=np.float32)
    for n in range(N):
        tab[n * VS : n * VS + V + 1] = tables[n]
        tab[n * VS + V + 1 : (n + 1) * VS] = emb0[:, n * E : (n + 1) * E]

    # per (core,batch): sort tokens by total count (== wordlen surrogate)
    special = x < 4                                    # [B, S]
    cnt_eff = np.where(special[..., None], 1, cnt)     # [B, S, 3]
    totc = np.where(special, 1, cnt.sum(-1))           # sort key [B, S]
    perm = np.argsort(totc, axis=1, kind="stable")     # sorted order -> orig pos
    cnt_sorted = np.take_along_axis(cnt_eff, perm[..., None], axis=1)  # [B,S,3]

    # shared K structure: K[r][n] = max over all batches of count at last rank of tile r
    Ksh = np.zeros((TILES, N), dtype=np.int64)
    for r in range(TILES):
        Ksh[r] = cnt_sorted[:, (r + 1) * 128 - 1, :].max(axis=0)
    Ksh = np.clip(Ksh, 1, L)

    # flat gather row ids per (b, s_orig, n, l): pad -> n*VS (zero row);
    # special -> first slot n*VS+V+1+x, count 1
    rows = ids + (np.arange(N) * VS)[None, None, :, None]          # [B,S,3,12]
    mask = np.arange(L)[None, None, None, :] < cnt_eff[..., None]
    rows = np.where(mask, rows, (np.arange(N) * VS)[None, None, :, None])
    sp_rows = (np.arange(N) * VS)[None, :] + V + 1 + x[..., None]  # [B,S,3]
    rows[special] = (np.arange(N)[None, :, None] * VS)             # zero all slots
    rows[special, :, 0] = sp_rows[special]

    pairs, jmax, prefix = _pair_schedule(Ksh)

    # device emission order: for bb, for j, for q in prefix[j] -> one offset col
    per_core = []
    for c in range(NCORES):
        offcols, rcp, pos = [], [], []
        for bb in range(BPC):
            b = c * BPC + bb
            pm = perm[b]
            for j in range(jmax):
                for q in range(prefix[j]):
                    r, n = pairs[q]
                    if Ksh[r][n] > j:
                        tokens = pm[r * 128 : (r + 1) * 128]
                        offcols.append(rows[b][tokens, n, j])
                    else:  # pad column (j==1 only): zero row of order n
                        offcols.append(np.full(128, n * VS, dtype=np.int64))
            for r in range(TILES):
                for n in range(N):
                    rcp.append(1.0 / cnt_sorted[b, r * 128 : (r + 1) * 128, n])
                pos.append(pm[r * 128 : (r + 1) * 128])
        per_core.append(
            dict(
                off=np.stack(offcols, axis=1).astype(np.int32),   # [128, ncols]
                rcp=np.stack(rcp, axis=1).astype(np.float32),     # [128, 2*16*3]
                pos=np.stack(pos, axis=1).astype(np.int32),       # [128, 2*16]
            )
        )

    # ---- host-folded weight norm ----
    # nrm[l,k] = ||v[l,:,:,k]||; wgt = g*v/nrm.  a-half used as-is, b-half
    # (sigmoid input) scaled by C^l; a-bias scaled C^-l, b-bias raw.
    nrm = np.sqrt(np.sum(conv_v * conv_v, axis=(1, 2)))            # [LYR, KC]
    wgt = conv_v * (conv_g / nrm)[:, None, None, :]                # [LYR, 2W, W, KC]
    scale_h = np.stack(
        [np.ones(LYR), C ** np.arange(LYR)], axis=1                # [LYR, 2]
    )
    # -> [i(128), l, h, k, ci, o(384)] contiguous per partition
    wv = wgt.reshape(LYR, 2, 384, 3, 128, KC)                      # l h o ci i k
    wv = wv * scale_h[:, :, None, None, None, None]
    wt_host = np.ascontiguousarray(
        wv.transpose(4, 0, 1, 5, 3, 2)                             # i l h k ci o
    ).reshape(128, LYR * 2 * KC * 3 * 384)
    import ml_dtypes
    wt_host = wt_host.astype(ml_dtypes.bfloat16)

    cb = conv_b.reshape(LYR, 6, 128).transpose(2, 0, 1).copy()     # [128, LYR, 6]
    cb[:, :, 0:3] *= (C ** (-np.arange(LYR)))[None, :, None]       # a-bias * C^-l
    cb_host = np.ascontiguousarray(cb.reshape(128, LYR * 6)).astype(np.float32)
    return tab, wt_host, cb_host, per_core, Ksh, perm


def _build(Ksh, repeat=1):
    nc = bacc.Bacc("TRN2", target_bir_lowering=False, debug=False)
    pairs, jmax, prefix = _pair_schedule(Ksh)
    ncols = BPC * sum(prefix)
    WTW = LYR * 2 * KC * 3 * 384  # 34560 bf16 per partition

    t_tab = nc.dram_tensor("tab", [3 * VS, E], mybir.dt.float32, kind="ExternalInput")
    t_off = nc.dram_tensor("off", [128, ncols], mybir.dt.int32, kind="ExternalInput")
    t_rcp = nc.dram_tensor("rcp", [128, BPC * TILES * N], mybir.dt.float32, kind="ExternalInput")
    t_pos = nc.dram_tensor("pos", [128, BPC * TILES], mybir.dt.int32, kind="ExternalInput")
    t_wt = nc.dram_tensor("wt", [128, WTW], mybir.dt.bfloat16, kind="ExternalInput")
    t_cb = nc.dram_tensor("cb", [128, LYR * 6], mybir.dt.float32, kind="ExternalInput")
    t_est = [
        nc.dram_tensor(f"e_st{i}", [S, W], mybir.dt.float32, kind="ExternalOutput")
        for i in range(BPC)
    ]
    t_ebst = [
        nc.dram_tensor(f"ebst{i}", [S, W], mybir.dt.bfloat16, kind="Internal")
        for i in range(BPC)
    ]
    t_h = nc.dram_tensor("h_out", [BPC, W, S], mybir.dt.float32, kind="ExternalOutput")

    HW_ = 2112  # stripe width: tokens at [32, 2080), halos at 31 / 2080

    from contextlib import nullcontext
    with tile.TileContext(nc) as tc, ExitStack() as ctx:
        consts = ctx.enter_context(tc.tile_pool(name="consts", bufs=1))
        gdp = ctx.enter_context(tc.tile_pool(name="gdp", bufs=2))
        asmp = ctx.enter_context(tc.tile_pool(name="asmp", bufs=4))
        bfac = ctx.enter_context(tc.tile_pool(name="bfac", bufs=2))
        hstr = ctx.enter_context(tc.tile_pool(name="hstr", bufs=3))
        sgp = ctx.enter_context(tc.tile_pool(name="sgp", bufs=6))
        hop = ctx.enter_context(tc.tile_pool(name="hop", bufs=4))
        psc = ctx.enter_context(tc.tile_pool(name="psc", bufs=3, space="PSUM"))

        off_t = consts.tile([128, ncols], mybir.dt.int32)
        nc.sync.dma_start(off_t[:], t_off.ap())
        rcp_t = consts.tile([128, BPC * TILES * N], mybir.dt.float32)
        nc.sync.dma_start(rcp_t[:], t_rcp.ap())
        pos_t = consts.tile([128, BPC * TILES], mybir.dt.int32)
        nc.sync.dma_start(pos_t[:], t_pos.ap())
        cb_t = consts.tile([128, LYR * 6], mybir.dt.float32)
        nc.sync.dma_start(cb_t[:], t_cb.ap())
        wt_t = consts.tile([128, WTW], mybir.dt.bfloat16)
        half = WTW // 2
        nc.sync.dma_start(wt_t[:, :half], t_wt.ap()[:, :half])
        nc.sync.dma_start(wt_t[:, half:], t_wt.ap()[:, half:])

        def wslice(l, h, k, ci, pj):
            base = (((l * 2 + h) * KC + k) * 3 + ci) * 384 + pj * 128
            return wt_t[:, base : base + 128]

        rep_ctx = tc.For_i(0, repeat, 1) if repeat > 1 else nullcontext()
        ctx.enter_context(rep_ctx)
        # ---------------- embedding phase (both batches) ----------------
        colidx = 0
        for bb in range(BPC):
            gA = gdp.tile([128, NP * E], mybir.dt.float32, name=f"gA_{bb}", tag="gA")
            gB = gdp.tile([128, NP * E], mybir.dt.float32, name=f"gB_{bb}", tag="gB")
            for j in range(jmax):
                P = prefix[j]
                dst = gA if j % 2 == 0 else gB
                nc.gpsimd.indirect_dma_start(
                    out=dst[:, : P * E],
                    out_offset=None,
                    in_=t_tab.ap(),
                    in_offset=bass.IndirectOffsetOnAxis(
                        ap=off_t[:, colidx : colidx + P], axis=0
                    ),
                    compute_op=(
                        mybir.AluOpType.bypass if j < 2 else mybir.AluOpType.add
                    ),
                )
                colidx += P
            bfa = bfac.tile([128, TILES * W], mybir.dt.bfloat16, name=f"bf_{bb}", tag="bfa")
            q_of = {rn: q for q, rn in enumerate(pairs)}
            for r in range(TILES):
                asm = asmp.tile([128, W], mybir.dt.float32, name=f"a_{bb}_{r}", tag="asm")
                for n in range(N):
                    q = q_of[(r, n)]
                    col = (bb * TILES + r) * N + n
                    nc.vector.tensor_add(
                        asm[:, n * E : (n + 1) * E],
                        gA[:, q * E : (q + 1) * E],
                        gB[:, q * E : (q + 1) * E],
                    )
                    nc.vector.tensor_scalar_mul(
                        asm[:, n * E : (n + 1) * E],
                        asm[:, n * E : (n + 1) * E],
                        rcp_t[:, col : col + 1],
                    )
                nc.vector.tensor_copy(bfa[:, r * W : (r + 1) * W], asm[:])
                nc.sync.dma_start(
                    t_est[bb].ap()[r * 128 : (r + 1) * 128, :], asm[:]
                )
            nc.gpsimd.indirect_dma_start(
                out=t_ebst[bb].ap(),
                out_offset=bass.IndirectOffsetOnAxis(
                    ap=pos_t[:, bb * TILES : (bb + 1) * TILES], axis=0
                ),
                in_=bfa[:],
                in_offset=None,
            )
        assert colidx == ncols

        # ---------------- conv phase per batch ----------------
        for bb in range(BPC):
            h0 = hstr.tile([128, N, HW_], mybir.dt.bfloat16, name=f"h0_{bb}", tag="hs")
            nc.vector.memset(h0[:, :, 31:32], 0.0)
            nc.vector.memset(h0[:, :, 2080:2081], 0.0)
            for n in range(N):
                nc.sync.dma_start(
                    h0[:, n, 32:2080],
                    t_ebst[bb].ap()[:, n * E : (n + 1) * E],
                    transpose=True,
                )
            hcur = h0
            for l in range(LYR):
                hnext = (
                    hstr.tile([128, N, HW_], mybir.dt.bfloat16, name=f"h{bb}_{l + 1}", tag="hs")
                    if l < LYR - 1
                    else None
                )
                if hnext is not None:
                    nc.vector.memset(hnext[:, :, 31:32], 0.0)
                    nc.vector.memset(hnext[:, :, 2080:2081], 0.0)
                for pj in range(3):
                    for nt in range(4):
                        ps_a = psc.tile([128, 512], mybir.dt.float32, space="PSUM",
                                        name=f"pa{bb}{l}{pj}{nt}", tag="psa")
                        ps_b = psc.tile([128, 512], mybir.dt.float32, space="PSUM",
                                        name=f"pq{bb}{l}{pj}{nt}", tag="psb")
                        for ci in range(3):
                            for k in range(KC):
                                rhs = hcur[:, ci, 32 + nt * 512 + k - 1 : 32 + nt * 512 + k + 511]
                                st = ci == 0 and k == 0
                                sp = ci == 2 and k == KC - 1
                                nc.tensor.matmul(
                                    ps_a[:], wslice(l, 0, k, ci, pj),
                                    rhs, start=st, stop=sp,
                                )
                                nc.tensor.matmul(
                                    ps_b[:], wslice(l, 1, k, ci, pj),
                                    rhs, start=st, stop=sp,
                                )
                        sig = sgp.tile([128, 512], mybir.dt.bfloat16,
                                       name=f"sg{bb}{l}{pj}{nt}", tag="sig")
                        nc.scalar.activation(
                            sig[:], ps_b[:], mybir.ActivationFunctionType.Sigmoid,
                            bias=cb_t[:, l * 6 + 3 + pj : l * 6 + 4 + pj], scale=1.0,
                        )
                        if hnext is not None:
                            glu = sgp.tile([128, 512], mybir.dt.bfloat16,
                                           name=f"gl{bb}{l}{pj}{nt}", tag="glu")
                            nc.vector.scalar_tensor_tensor(
                                glu[:], ps_a[:], cb_t[:, l * 6 + pj : l * 6 + pj + 1], sig[:],
                                op0=mybir.AluOpType.add, op1=mybir.AluOpType.mult,
                            )
                            nc.vector.tensor_add(
                                hnext[:, pj, 32 + nt * 512 : 32 + (nt + 1) * 512],
                                glu[:],
                                hcur[:, pj, 32 + nt * 512 : 32 + (nt + 1) * 512],
                            )
                        else:
                            # last layer: h_out = C^5*(glu + hcur) computed in fp32
                            glu = sgp.tile([128, 512], mybir.dt.float32,
                                           name=f"gl{bb}{l}{pj}{nt}", tag="gluf")
                            nc.vector.scalar_tensor_tensor(
                                glu[:], ps_a[:], cb_t[:, l * 6 + pj : l * 6 + pj + 1], sig[:],
                                op0=mybir.AluOpType.add, op1=mybir.AluOpType.mult,
                            )
                            ho = hop.tile([128, 512], mybir.dt.float32, name=f"ho{bb}{pj}{nt}", tag="ho")
                            nc.vector.scalar_tensor_tensor(
                                ho[:], hcur[:, pj, 32 + nt * 512 : 32 + (nt + 1) * 512],
                                1.0, glu[:],
                                op0=mybir.AluOpType.mult, op1=mybir.AluOpType.add,
                            )
                            hs = hop.tile([128, 512], mybir.dt.float32,
                                          name=f"hs{bb}{pj}{nt}", tag="hsc")
                            nc.vector.tensor_scalar_mul(hs[:], ho[:], C**LYR)
                            nc.sync.dma_start(
                                t_h.ap()[bb][pj * 128 : (pj + 1) * 128,
                                             nt * 512 : (nt + 1) * 512],
                                hs[:],
                            )
                hcur = hnext if hnext is not None else hcur
    nc.compile()
    return nc


_CACHE = {}


def _run(inputs, trace=False, repeat=1):
    tab, wt_host, cb_host, per_core, Ksh, perm = _host_prep(inputs)
    key = (Ksh.tobytes(), repeat)
    if key not in _CACHE:
        _CACHE[key] = _build(Ksh, repeat=repeat)
    nc = _CACHE[key]
    in_maps = [
        dict(tab=tab, off=pc["off"], rcp=pc["rcp"], pos=pc["pos"],
             wt=wt_host, cb=cb_host)
        for pc in per_core
    ]
    res = run_bass_kernel_spmd(nc, in_maps, core_ids=list(range(NCORES)), trace=trace)
    h = np.concatenate([r["h_out"] for r in res.results], axis=0)        # [16, 384, 2048]
    e = np.empty((B, W, S), dtype=np.float32)
    for c in range(NCORES):
        for i in range(BPC):
            b = c * BPC + i
            es = np.empty((S, W), dtype=np.float32)
            es[perm[b]] = res.results[c][f"e_st{i}"]     # unsort rows
            e[b] = es.T
    return (h.astype(np.float32), np.ascontiguousarray(e).astype(np.float32)), res


def kernel(**inputs):
    out, _ = _run(inputs)
    return out
